# revision 5
# baseline (speedup 1.0000x reference)
"""AttentionBlock kernel for 8 Trainium2 NeuronCores.

Sharding: one (batch, head) pair per core (B=2 x H=4 = 8 cores).
Each core computes, for its (b, h):
    qT/kT = (w_q/k^T @ x_b) + bias        [64, S]   (S padded to 2816)
    v     = x_b^T @ w_v                   [S, 64]  (+ ones column -> [S, 65])
    S^T[j, i] = sum_d k[j,d] q[i,d]                (22 j-tiles of 128)
    E = exp(S^T * 0.125 - 3)                       (ScalarE, from PSUM)
    resT[d, i] = sum_j v_aug[j, d] E[j, i]         (PSUM accumulation, 65 rows;
                                                    row 64 = softmax denominator l)
    outT[c, i] = sum_d w_out[d, c] resT[d, i]      [256, S] (unnormalized)
Host: out_b = sum_h (outT / l + (b_v @ w_out_h)[:, None]) + b_out[:, None] + x_b.

The -3 bias and missing max-subtraction cancel in the normalization; score
scale is 1/sqrt(64) = 0.125. All math fp32.
"""

import numpy as np

C = 256
S = 2744
SP = 2816  # 22 * 128
H = 4
DK = 64
NT = 22  # j tiles of 128
SVALID_LAST = S - 21 * 128  # 56 valid rows in last j-tile

# i blocks (query positions): only valid range [0, 2744)
IBLOCKS = [(0, 512), (512, 512), (1024, 512), (1536, 512), (2048, 512), (2560, 184)]
# s blocks for the qk projection: full padded range [0, 2816)
SBLOCKS = [(0, 512), (512, 512), (1024, 512), (1536, 512), (2048, 512), (2560, 256)]

_NC = None


def _build():
    from contextlib import ExitStack

    import concourse.bacc as bacc
    import concourse.tile as tile
    from concourse import mybir

    f32 = mybir.dt.float32
    Exp = mybir.ActivationFunctionType.Exp

    nc = bacc.Bacc("TRN2", target_bir_lowering=False)

    xT = nc.dram_tensor("xT", [C, S], f32, kind="ExternalInput")
    wq = nc.dram_tensor("wq", [C, DK], f32, kind="ExternalInput")
    wk = nc.dram_tensor("wk", [C, DK], f32, kind="ExternalInput")
    wv = nc.dram_tensor("wv", [C, DK], f32, kind="ExternalInput")
    bq = nc.dram_tensor("bq", [DK, 1], f32, kind="ExternalInput")
    bk = nc.dram_tensor("bk", [DK, 1], f32, kind="ExternalInput")
    wo = nc.dram_tensor("wo", [DK, C], f32, kind="ExternalInput")

    out = nc.dram_tensor("out", [C, S], f32, kind="ExternalOutput")
    lsum = nc.dram_tensor("lsum", [1, S], f32, kind="ExternalOutput")

    with tile.TileContext(nc) as tc, ExitStack() as ctx:
        consts = ctx.enter_context(tc.tile_pool(name="consts", bufs=1))
        big = ctx.enter_context(tc.tile_pool(name="big", bufs=1))
        expp = ctx.enter_context(tc.tile_pool(name="expp", bufs=3))
        resp = ctx.enter_context(tc.tile_pool(name="resp", bufs=2))
        outp = ctx.enter_context(tc.tile_pool(name="outp", bufs=2))
        scp = ctx.enter_context(tc.tile_pool(name="scp", bufs=2, space="PSUM"))
        psp = ctx.enter_context(tc.tile_pool(name="psp", bufs=4, space="PSUM"))

        # ---- weights / constants in SBUF ----
        wq_sb = consts.tile([128, 2, DK], f32)
        wk_sb = consts.tile([128, 2, DK], f32)
        wv_sb = consts.tile([128, 2, DK], f32)
        for w_sb, w_dram in ((wq_sb, wq), (wk_sb, wk), (wv_sb, wv)):
            nc.sync.dma_start(out=w_sb, in_=w_dram.rearrange("(c p) d -> p c d", p=128))
        wo_sb = consts.tile([DK, C], f32)
        nc.sync.dma_start(out=wo_sb, in_=wo[:, :])
        bq_sb = consts.tile([DK, 1], f32)
        nc.sync.dma_start(out=bq_sb, in_=bq[:, :])
        bk_sb = consts.tile([DK, 1], f32)
        nc.sync.dma_start(out=bk_sb, in_=bk[:, :])
        ebias_sb = consts.tile([128, 1], f32)
        nc.vector.memset(ebias_sb, -3.0)

        # ---- x in SBUF: [128, chunk, SP], padded columns zeroed ----
        x_sb = big.tile([128, 2, SP], f32)
        nc.vector.memset(x_sb[:, :, S:SP], 0.0)
        for cc in range(2):
            for off, w in SBLOCKS:
                wv_ = min(w, S - off) if off < S else 0
                if wv_ > 0:
                    nc.sync.dma_start(
                        out=x_sb[:, cc, off : off + wv_],
                        in_=xT[cc * 128 : (cc + 1) * 128, off : off + wv_],
                    )

        # ---- q/k projections -> qT_sb/kT_sb [64, SP] (bias added) ----
        qT_sb = big.tile([DK, SP], f32)
        kT_sb = big.tile([DK, SP], f32)
        for off, w in SBLOCKS:
            psq = psp.tile([DK, 512], f32, tag="ps", name="psq")
            for cc in range(2):
                nc.tensor.matmul(
                    psq[:, :w],
                    lhsT=wq_sb[:, cc, :],
                    rhs=x_sb[:, cc, off : off + w],
                    start=(cc == 0),
                    stop=(cc == 1),
                )
            nc.vector.tensor_scalar_add(qT_sb[:, off : off + w], psq[:, :w], bq_sb)
            psk = psp.tile([DK, 512], f32, tag="ps", name="psk")
            for cc in range(2):
                nc.tensor.matmul(
                    psk[:, :w],
                    lhsT=wk_sb[:, cc, :],
                    rhs=x_sb[:, cc, off : off + w],
                    start=(cc == 0),
                    stop=(cc == 1),
                )
            nc.vector.tensor_scalar_add(kT_sb[:, off : off + w], psk[:, :w], bk_sb)

        # ---- v projection -> v_sb [128, NT, 65]; col 64 = ones (0 in pad rows) ----
        v_sb = big.tile([128, NT, DK + 1], f32)
        nc.vector.memset(v_sb[:, : NT - 1, DK : DK + 1], 1.0)
        nc.vector.memset(v_sb[:, NT - 1, DK : DK + 1], 0.0)
        nc.vector.memset(v_sb[:SVALID_LAST, NT - 1, DK : DK + 1], 1.0)
        for t in range(NT):
            psv = psp.tile([128, DK], f32, tag="ps", name="psv")
            for cc in range(2):
                nc.tensor.matmul(
                    psv,
                    lhsT=x_sb[:, cc, t * 128 : (t + 1) * 128],
                    rhs=wv_sb[:, cc, :],
                    start=(cc == 0),
                    stop=(cc == 1),
                )
            nc.vector.tensor_copy(v_sb[:, t, :DK], psv)

        # ---- main attention loop ----
        NG = NT // 2  # groups of 2 j-tiles per exp op
        pending_tail = None
        for ioff, iw in IBLOCKS:
            pv = psp.tile([DK + 1, 512], f32, tag="ps", name="pv")
            prev_pv = None  # (ex tile, group index) pending PV matmuls
            for g in range(NG):
                sc = scp.tile([128, 1024], f32, tag="sc", name="sc")
                for u in range(2):
                    t = 2 * g + u
                    nc.tensor.matmul(
                        sc[:, u * iw : (u + 1) * iw],
                        lhsT=kT_sb[:, t * 128 : (t + 1) * 128],
                        rhs=qT_sb[:, ioff : ioff + iw],
                        start=True,
                        stop=True,
                    )
                ex = expp.tile([128, 1024], f32, tag="ex", name="ex")
                nc.scalar.activation(
                    out=ex[:, : 2 * iw],
                    in_=sc[:, : 2 * iw],
                    func=Exp,
                    bias=ebias_sb,
                    scale=0.125,
                )
                if g == 1 and pending_tail is not None:
                    pending_tail()
                    pending_tail = None
                if prev_pv is not None:
                    pex, pg = prev_pv
                    for u in range(2):
                        t = 2 * pg + u
                        nc.tensor.matmul(
                            pv[:, :iw],
                            lhsT=v_sb[:, t, :],
                            rhs=pex[:, u * iw : (u + 1) * iw],
                            start=(t == 0),
                            stop=False,
                        )
                prev_pv = (ex, g)
            pex, pg = prev_pv
            for u in range(2):
                t = 2 * pg + u
                nc.tensor.matmul(
                    pv[:, :iw],
                    lhsT=v_sb[:, t, :],
                    rhs=pex[:, u * iw : (u + 1) * iw],
                    start=False,
                    stop=(t == NT - 1),
                )
            res_sb = resp.tile([DK + 1, 512], f32, tag="res", name="res_sb")
            nc.vector.tensor_copy(res_sb[:, :iw], pv[:, :iw])
            nc.sync.dma_start(
                out=lsum[0:1, ioff : ioff + iw], in_=res_sb[DK : DK + 1, :iw]
            )

            def tail(ioff=ioff, iw=iw, res_sb=res_sb):
                for cc in range(2):
                    po = psp.tile([128, 512], f32, tag="ps", name="po")
                    nc.tensor.matmul(
                        po[:, :iw],
                        lhsT=wo_sb[:, cc * 128 : (cc + 1) * 128],
                        rhs=res_sb[:DK, :iw],
                        start=True,
                        stop=True,
                    )
                    ob = outp.tile([128, 512], f32, tag="ob", name="ob")
                    nc.vector.tensor_copy(ob[:, :iw], po[:, :iw])
                    nc.sync.dma_start(
                        out=out[cc * 128 : (cc + 1) * 128, ioff : ioff + iw],
                        in_=ob[:, :iw],
                    )

            pending_tail = tail
        pending_tail()

    nc.compile()
    return nc


def _get_nc():
    global _NC
    if _NC is None:
        _NC = _build()
    return _NC


def kernel(x, w_proj, b_proj, w_out, b_out):
    from concourse.bass_utils import run_bass_kernel_spmd

    x = np.asarray(x, dtype=np.float32)
    w_proj = np.asarray(w_proj, dtype=np.float32)
    b_proj = np.asarray(b_proj, dtype=np.float32)
    w_out = np.asarray(w_out, dtype=np.float32)
    b_out = np.asarray(b_out, dtype=np.float32)

    B = x.shape[0]
    nc = _get_nc()

    in_maps = []
    for core in range(8):
        b, h = divmod(core, H)
        base = h * 3 * DK
        in_maps.append(
            {
                "xT": np.ascontiguousarray(x[b].reshape(C, S)),
                "wq": np.ascontiguousarray(w_proj[:, base : base + DK]),
                "wk": np.ascontiguousarray(w_proj[:, base + DK : base + 2 * DK]),
                "wv": np.ascontiguousarray(w_proj[:, base + 2 * DK : base + 3 * DK]),
                "bq": np.ascontiguousarray(b_proj[base : base + DK].reshape(DK, 1)),
                "bk": np.ascontiguousarray(
                    b_proj[base + DK : base + 2 * DK].reshape(DK, 1)
                ),
                "wo": np.ascontiguousarray(w_out[h * DK : (h + 1) * DK, :]),
            }
        )

    res = run_bass_kernel_spmd(nc, in_maps, list(range(8)))

    outs = np.zeros((B, C, S), dtype=np.float32)
    for b in range(B):
        acc = x[b].reshape(C, S).astype(np.float32) + b_out[:, None]
        for h in range(H):
            core = b * H + h
            dev_o = res.results[core]["out"]  # [C, S] unnormalized
            l = res.results[core]["lsum"]  # [1, S]
            bv = b_proj[h * 3 * DK + 2 * DK : h * 3 * DK + 3 * DK]
            corr = bv @ w_out[h * DK : (h + 1) * DK, :]  # [C]
            acc = acc + dev_o / l + corr[:, None]
        outs[b] = acc
    return outs.reshape(B, C, 14, 14, 14)


# revision 8
# speedup vs baseline: 1.6097x; 1.6097x over previous
"""AttentionBlock kernel for 8 Trainium2 NeuronCores.

Sharding: one (batch, head) pair per core (B=2 x H=4 = 8 cores).
Each core computes, for its (b, h):
    qT/kT = (w_q/k^T @ x_b) + bias        [64, S]   (S padded to 2816)
    v     = x_b^T @ w_v                   [S, 64]  (+ ones column -> [S, 65])
    S^T[j, i] = sum_d k[j,d] q[i,d]                (22 j-tiles of 128)
    E = exp(S^T * 0.125 - 3)                       (ScalarE, from PSUM)
    resT[d, i] = sum_j v_aug[j, d] E[j, i]         (PSUM accumulation, 65 rows;
                                                    row 64 = softmax denominator l)
    outT[c, i] = sum_d w_out[d, c] resT[d, i]      [256, S] (unnormalized)
Host: out_b = sum_h (outT / l + (b_v @ w_out_h)[:, None]) + b_out[:, None] + x_b.

The -3 bias and missing max-subtraction cancel in the normalization; score
scale is 1/sqrt(64) = 0.125.

Matmul operands use float32r (single-pass PE fp32, ~TF32 precision, ~3x the
throughput of 2-pass fp32); PSUM accumulation stays fp32.
"""

import numpy as np

C = 256
S = 2744
SP = 2816  # 22 * 128
H = 4
DK = 64
NT = 22  # j tiles of 128
SVALID_LAST = S - 21 * 128  # 56 valid rows in last j-tile

# i blocks (query positions): only valid range [0, 2744)
IBLOCKS = [(0, 512), (512, 512), (1024, 512), (1536, 512), (2048, 512), (2560, 184)]
# s blocks for the qk projection: full padded range [0, 2816)
SBLOCKS = [(0, 512), (512, 512), (1024, 512), (1536, 512), (2048, 512), (2560, 256)]

_NC = None


def _build():
    from contextlib import ExitStack

    import concourse.bacc as bacc
    import concourse.tile as tile
    from concourse import mybir

    f32 = mybir.dt.float32
    fr = mybir.dt.float32r
    Exp = mybir.ActivationFunctionType.Exp

    nc = bacc.Bacc("TRN2", target_bir_lowering=False)

    xT = nc.dram_tensor("xT", [C, S], f32, kind="ExternalInput")
    wq = nc.dram_tensor("wq", [C, DK], f32, kind="ExternalInput")
    wk = nc.dram_tensor("wk", [C, DK], f32, kind="ExternalInput")
    wv = nc.dram_tensor("wv", [C, DK], f32, kind="ExternalInput")
    bq = nc.dram_tensor("bq", [DK, 1], f32, kind="ExternalInput")
    bk = nc.dram_tensor("bk", [DK, 1], f32, kind="ExternalInput")
    wo = nc.dram_tensor("wo", [DK, C], f32, kind="ExternalInput")

    out = nc.dram_tensor("out", [C, S], f32, kind="ExternalOutput")
    lsum = nc.dram_tensor("lsum", [1, S], f32, kind="ExternalOutput")

    with tile.TileContext(nc) as tc, ExitStack() as ctx:
        consts = ctx.enter_context(tc.tile_pool(name="consts", bufs=1))
        big = ctx.enter_context(tc.tile_pool(name="big", bufs=1))
        expp = ctx.enter_context(tc.tile_pool(name="expp", bufs=3))
        resp = ctx.enter_context(tc.tile_pool(name="resp", bufs=2))
        outp = ctx.enter_context(tc.tile_pool(name="outp", bufs=2))
        scp = ctx.enter_context(tc.tile_pool(name="scp", bufs=2, space="PSUM"))
        psp = ctx.enter_context(tc.tile_pool(name="psp", bufs=4, space="PSUM"))

        # ---- weights / constants in SBUF (staged fp32, converted to f32r) ----
        w_stage = consts.tile([128, 2, 3 * DK], f32)
        for idx, w_dram in enumerate((wq, wk, wv)):
            nc.sync.dma_start(
                out=w_stage[:, :, idx * DK : (idx + 1) * DK],
                in_=w_dram.rearrange("(c p) d -> p c d", p=128),
            )
        w_sb = consts.tile([128, 2, 3 * DK], fr)
        nc.vector.tensor_copy(w_sb, w_stage)

        def wslice(idx, cc):
            return w_sb[:, cc, idx * DK : (idx + 1) * DK]

        wo_stage = consts.tile([DK, C], f32)
        nc.sync.dma_start(out=wo_stage, in_=wo[:, :])
        wo_sb = consts.tile([DK, C], fr)
        nc.vector.tensor_copy(wo_sb, wo_stage)

        bq_sb = consts.tile([DK, 1], f32)
        nc.sync.dma_start(out=bq_sb, in_=bq[:, :])
        bk_sb = consts.tile([DK, 1], f32)
        nc.sync.dma_start(out=bk_sb, in_=bk[:, :])
        ebias_sb = consts.tile([128, 1], f32)
        nc.vector.memset(ebias_sb, -3.0)

        # ---- x in SBUF: staged fp32 then converted to f32r ----
        x_stage = big.tile([128, 2, SP], f32)
        nc.vector.memset(x_stage[:, :, S:SP], 0.0)
        for cc in range(2):
            for off, w in SBLOCKS:
                wv_ = min(w, S - off) if off < S else 0
                if wv_ > 0:
                    nc.sync.dma_start(
                        out=x_stage[:, cc, off : off + wv_],
                        in_=xT[cc * 128 : (cc + 1) * 128, off : off + wv_],
                    )
        x_sb = big.tile([128, 2, SP], fr)
        for off, w in SBLOCKS:
            nc.vector.tensor_copy(x_sb[:, :, off : off + w], x_stage[:, :, off : off + w])

        # ---- q/k projections -> qT_sb/kT_sb [64, SP] f32r (bias added) ----
        qT_sb = big.tile([DK, SP], fr)
        kT_sb = big.tile([DK, SP], fr)
        for off, w in SBLOCKS:
            psq = psp.tile([DK, 512], f32, tag="ps", name="psq")
            for cc in range(2):
                nc.tensor.matmul(
                    psq[:, :w],
                    lhsT=wslice(0, cc),
                    rhs=x_sb[:, cc, off : off + w],
                    start=(cc == 0),
                    stop=(cc == 1),
                )
            nc.vector.tensor_scalar_add(qT_sb[:, off : off + w], psq[:, :w], bq_sb)
            psk = psp.tile([DK, 512], f32, tag="ps", name="psk")
            for cc in range(2):
                nc.tensor.matmul(
                    psk[:, :w],
                    lhsT=wslice(1, cc),
                    rhs=x_sb[:, cc, off : off + w],
                    start=(cc == 0),
                    stop=(cc == 1),
                )
            nc.vector.tensor_scalar_add(kT_sb[:, off : off + w], psk[:, :w], bk_sb)

        # ---- v projection -> v_sb [128, NT, 65] f32r; col 64 = ones ----
        v_sb = big.tile([128, NT, DK + 1], fr)
        ones_f = consts.tile([128, NT, 1], f32)
        nc.vector.memset(ones_f, 1.0)
        nc.vector.memset(ones_f[:, NT - 1, :], 0.0)
        nc.vector.memset(ones_f[:SVALID_LAST, NT - 1, :], 1.0)
        nc.vector.tensor_copy(v_sb[:, :, DK : DK + 1], ones_f)
        for t in range(NT):
            psv = psp.tile([128, DK], f32, tag="ps", name="psv")
            for cc in range(2):
                nc.tensor.matmul(
                    psv,
                    lhsT=x_sb[:, cc, t * 128 : (t + 1) * 128],
                    rhs=wslice(2, cc),
                    start=(cc == 0),
                    stop=(cc == 1),
                )
            nc.vector.tensor_copy(v_sb[:, t, :DK], psv)

        # ---- main attention loop ----
        NG = NT // 2  # groups of 2 j-tiles per exp op
        pending_tail = None
        for ioff, iw in IBLOCKS:
            pv = psp.tile([DK + 1, 512], f32, tag="ps", name="pv")
            prev_pv = None  # (ex tile, group index) pending PV matmuls
            for g in range(NG):
                sc = scp.tile([128, 1024], f32, tag="sc", name="sc")
                for u in range(2):
                    t = 2 * g + u
                    nc.tensor.matmul(
                        sc[:, u * iw : (u + 1) * iw],
                        lhsT=kT_sb[:, t * 128 : (t + 1) * 128],
                        rhs=qT_sb[:, ioff : ioff + iw],
                        start=True,
                        stop=True,
                    )
                ex = expp.tile([128, 1024], fr, tag="ex", name="ex")
                nc.scalar.activation(
                    out=ex[:, : 2 * iw],
                    in_=sc[:, : 2 * iw],
                    func=Exp,
                    bias=ebias_sb,
                    scale=0.125,
                )
                if g == 1 and pending_tail is not None:
                    pending_tail()
                    pending_tail = None
                if prev_pv is not None:
                    pex, pg = prev_pv
                    for u in range(2):
                        t = 2 * pg + u
                        nc.tensor.matmul(
                            pv[:, :iw],
                            lhsT=v_sb[:, t, :],
                            rhs=pex[:, u * iw : (u + 1) * iw],
                            start=(t == 0),
                            stop=False,
                        )
                prev_pv = (ex, g)
            pex, pg = prev_pv
            for u in range(2):
                t = 2 * pg + u
                nc.tensor.matmul(
                    pv[:, :iw],
                    lhsT=v_sb[:, t, :],
                    rhs=pex[:, u * iw : (u + 1) * iw],
                    start=False,
                    stop=(t == NT - 1),
                )
            res_sb = resp.tile([DK + 1, 512], fr, tag="res", name="res_sb")
            nc.vector.tensor_copy(res_sb[:, :iw], pv[:, :iw])
            nc.sync.dma_start(
                out=lsum[0:1, ioff : ioff + iw],
                in_=res_sb[DK : DK + 1, :iw].bitcast(f32),
            )

            def tail(ioff=ioff, iw=iw, res_sb=res_sb):
                for cc in range(2):
                    po = psp.tile([128, 512], f32, tag="ps", name="po")
                    nc.tensor.matmul(
                        po[:, :iw],
                        lhsT=wo_sb[:, cc * 128 : (cc + 1) * 128],
                        rhs=res_sb[:DK, :iw],
                        start=True,
                        stop=True,
                    )
                    ob = outp.tile([128, 512], f32, tag="ob", name="ob")
                    nc.vector.tensor_copy(ob[:, :iw], po[:, :iw])
                    nc.sync.dma_start(
                        out=out[cc * 128 : (cc + 1) * 128, ioff : ioff + iw],
                        in_=ob[:, :iw],
                    )

            pending_tail = tail
        pending_tail()

    nc.compile()
    return nc


def _get_nc():
    global _NC
    if _NC is None:
        _NC = _build()
    return _NC


def kernel(x, w_proj, b_proj, w_out, b_out):
    from concourse.bass_utils import run_bass_kernel_spmd

    x = np.asarray(x, dtype=np.float32)
    w_proj = np.asarray(w_proj, dtype=np.float32)
    b_proj = np.asarray(b_proj, dtype=np.float32)
    w_out = np.asarray(w_out, dtype=np.float32)
    b_out = np.asarray(b_out, dtype=np.float32)

    B = x.shape[0]
    nc = _get_nc()

    in_maps = []
    for core in range(8):
        b, h = divmod(core, H)
        base = h * 3 * DK
        in_maps.append(
            {
                "xT": np.ascontiguousarray(x[b].reshape(C, S)),
                "wq": np.ascontiguousarray(w_proj[:, base : base + DK]),
                "wk": np.ascontiguousarray(w_proj[:, base + DK : base + 2 * DK]),
                "wv": np.ascontiguousarray(w_proj[:, base + 2 * DK : base + 3 * DK]),
                "bq": np.ascontiguousarray(b_proj[base : base + DK].reshape(DK, 1)),
                "bk": np.ascontiguousarray(
                    b_proj[base + DK : base + 2 * DK].reshape(DK, 1)
                ),
                "wo": np.ascontiguousarray(w_out[h * DK : (h + 1) * DK, :]),
            }
        )

    res = run_bass_kernel_spmd(nc, in_maps, list(range(8)))

    outs = np.zeros((B, C, S), dtype=np.float32)
    for b in range(B):
        acc = x[b].reshape(C, S).astype(np.float32) + b_out[:, None]
        for h in range(H):
            core = b * H + h
            dev_o = res.results[core]["out"]  # [C, S] unnormalized
            l = res.results[core]["lsum"]  # [1, S]
            bv = b_proj[h * 3 * DK + 2 * DK : h * 3 * DK + 3 * DK]
            corr = bv @ w_out[h * DK : (h + 1) * DK, :]  # [C]
            acc = acc + dev_o / l + corr[:, None]
        outs[b] = acc
    return outs.reshape(B, C, 14, 14, 14)


# revision 9
# speedup vs baseline: 2.4346x; 1.5125x over previous
"""AttentionBlock kernel for 8 Trainium2 NeuronCores.

Sharding: one (batch, head) pair per core (B=2 x H=4 = 8 cores).
Each core computes, for its (b, h):
    qT/kT = (w_q/k^T @ x_b) + bias        [64, S]   (S padded to 2816)
    v     = x_b^T @ w_v                   [S, 64]  (+ ones column -> [S, 65])
    S^T[j, i] = sum_d k[j,d] q[i,d]                (22 j-tiles of 128)
    E = exp(S^T * 0.125 - 3)                       (ScalarE, from PSUM)
    resT[d, i] = sum_j v_aug[j, d] E[j, i]         (PSUM accumulation, 65 rows;
                                                    row 64 = softmax denominator l)
    outT[c, i] = sum_d w_out[d, c] resT[d, i]      [256, S] (unnormalized)
Host: out_b = sum_h (outT / l + (b_v @ w_out_h)[:, None]) + b_out[:, None] + x_b.

The -3 bias and missing max-subtraction cancel in the normalization; score
scale is 1/sqrt(64) = 0.125.

Matmul operands use float32r (single-pass PE fp32, ~TF32 precision, ~3x the
throughput of 2-pass fp32); PSUM accumulation stays fp32.
"""

import numpy as np

C = 256
S = 2744
SP = 2816  # 22 * 128
H = 4
DK = 64
NT = 22  # j tiles of 128
SVALID_LAST = S - 21 * 128  # 56 valid rows in last j-tile

# i blocks (query positions): only valid range [0, 2744)
IBLOCKS = [(0, 512), (512, 512), (1024, 512), (1536, 512), (2048, 512), (2560, 184)]
# s blocks for the qk projection: full padded range [0, 2816)
SBLOCKS = [(0, 512), (512, 512), (1024, 512), (1536, 512), (2048, 512), (2560, 256)]

_NC = None


def _build():
    from contextlib import ExitStack

    import concourse.bacc as bacc
    import concourse.tile as tile
    from concourse import mybir

    f32 = mybir.dt.float32
    fr = mybir.dt.float32r
    f16 = mybir.dt.float16
    Exp = mybir.ActivationFunctionType.Exp

    nc = bacc.Bacc("TRN2", target_bir_lowering=False)

    xT = nc.dram_tensor("xT", [C, S], f32, kind="ExternalInput")
    wq = nc.dram_tensor("wq", [C, DK], f32, kind="ExternalInput")
    wk = nc.dram_tensor("wk", [C, DK], f32, kind="ExternalInput")
    wv = nc.dram_tensor("wv", [C, DK], f32, kind="ExternalInput")
    bq = nc.dram_tensor("bq", [DK, 1], f32, kind="ExternalInput")
    bk = nc.dram_tensor("bk", [DK, 1], f32, kind="ExternalInput")
    wo = nc.dram_tensor("wo", [DK, C], f32, kind="ExternalInput")

    out = nc.dram_tensor("out", [C, S], f32, kind="ExternalOutput")
    lsum = nc.dram_tensor("lsum", [1, S], f32, kind="ExternalOutput")

    with tile.TileContext(nc) as tc, ExitStack() as ctx:
        consts = ctx.enter_context(tc.tile_pool(name="consts", bufs=1))
        big = ctx.enter_context(tc.tile_pool(name="big", bufs=1))
        expp = ctx.enter_context(tc.tile_pool(name="expp", bufs=3))
        resp = ctx.enter_context(tc.tile_pool(name="resp", bufs=2))
        outp = ctx.enter_context(tc.tile_pool(name="outp", bufs=2))
        scp = ctx.enter_context(tc.tile_pool(name="scp", bufs=2, space="PSUM"))
        psp = ctx.enter_context(tc.tile_pool(name="psp", bufs=4, space="PSUM"))

        # ---- weights / constants in SBUF (staged fp32, converted to f32r) ----
        w_stage = consts.tile([128, 2, 3 * DK], f32)
        for idx, w_dram in enumerate((wq, wk, wv)):
            nc.sync.dma_start(
                out=w_stage[:, :, idx * DK : (idx + 1) * DK],
                in_=w_dram.rearrange("(c p) d -> p c d", p=128),
            )
        w_sb = consts.tile([128, 2, 3 * DK], f16)
        nc.vector.tensor_copy(w_sb, w_stage)

        def wslice(idx, cc):
            return w_sb[:, cc, idx * DK : (idx + 1) * DK]

        wo_stage = consts.tile([DK, C], f32)
        nc.sync.dma_start(out=wo_stage, in_=wo[:, :])
        wo_sb = consts.tile([DK, C], fr)
        nc.vector.tensor_copy(wo_sb, wo_stage)

        bq_sb = consts.tile([DK, 1], f32)
        nc.sync.dma_start(out=bq_sb, in_=bq[:, :])
        bk_sb = consts.tile([DK, 1], f32)
        nc.sync.dma_start(out=bk_sb, in_=bk[:, :])
        ebias_sb = consts.tile([128, 1], f32)
        nc.vector.memset(ebias_sb, -3.0)

        # ---- x in SBUF: staged fp32 then converted to f32r ----
        x_stage = big.tile([128, 2, SP], f32)
        nc.vector.memset(x_stage[:, :, S:SP], 0.0)
        for cc in range(2):
            for off, w in SBLOCKS:
                wv_ = min(w, S - off) if off < S else 0
                if wv_ > 0:
                    nc.sync.dma_start(
                        out=x_stage[:, cc, off : off + wv_],
                        in_=xT[cc * 128 : (cc + 1) * 128, off : off + wv_],
                    )
        x_sb = big.tile([128, 2, SP], f16)
        for off, w in SBLOCKS:
            nc.vector.tensor_copy(x_sb[:, :, off : off + w], x_stage[:, :, off : off + w])

        # ---- q/k projections -> qT_sb/kT_sb [64, SP] f32r (bias added) ----
        qT_sb = big.tile([DK, SP], f16)
        kT_sb = big.tile([DK, SP], f16)
        for off, w in SBLOCKS:
            psq = psp.tile([DK, 512], f32, tag="ps", name="psq")
            for cc in range(2):
                nc.tensor.matmul(
                    psq[:, :w],
                    lhsT=wslice(0, cc),
                    rhs=x_sb[:, cc, off : off + w],
                    start=(cc == 0),
                    stop=(cc == 1),
                )
            nc.vector.tensor_scalar_add(qT_sb[:, off : off + w], psq[:, :w], bq_sb)
            psk = psp.tile([DK, 512], f32, tag="ps", name="psk")
            for cc in range(2):
                nc.tensor.matmul(
                    psk[:, :w],
                    lhsT=wslice(1, cc),
                    rhs=x_sb[:, cc, off : off + w],
                    start=(cc == 0),
                    stop=(cc == 1),
                )
            nc.vector.tensor_scalar_add(kT_sb[:, off : off + w], psk[:, :w], bk_sb)

        # ---- v projection -> v_sb [128, NT, 65] f32r; col 64 = ones ----
        v_sb = big.tile([128, NT, DK + 1], f16)
        nc.vector.memset(v_sb[:, : NT - 1, DK : DK + 1], 1.0)
        nc.vector.memset(v_sb[:, NT - 1, DK : DK + 1], 0.0)
        nc.vector.memset(v_sb[:SVALID_LAST, NT - 1, DK : DK + 1], 1.0)
        for t in range(NT):
            psv = psp.tile([128, DK], f32, tag="ps", name="psv")
            for cc in range(2):
                nc.tensor.matmul(
                    psv,
                    lhsT=x_sb[:, cc, t * 128 : (t + 1) * 128],
                    rhs=wslice(2, cc),
                    start=(cc == 0),
                    stop=(cc == 1),
                )
            nc.vector.tensor_copy(v_sb[:, t, :DK], psv)

        # ---- main attention loop ----
        NG = NT // 2  # groups of 2 j-tiles per exp op
        pending_tail = None
        for ioff, iw in IBLOCKS:
            pv = psp.tile([DK + 1, 512], f32, tag="ps", name="pv")
            prev_pv = None  # (ex tile, group index) pending PV matmuls
            for g in range(NG):
                sc = scp.tile([128, 1024], f32, tag="sc", name="sc")
                for u in range(2):
                    t = 2 * g + u
                    nc.tensor.matmul(
                        sc[:, u * iw : (u + 1) * iw],
                        lhsT=kT_sb[:, t * 128 : (t + 1) * 128],
                        rhs=qT_sb[:, ioff : ioff + iw],
                        start=True,
                        stop=True,
                    )
                ex = expp.tile([128, 1024], f16, tag="ex", name="ex")
                nc.scalar.activation(
                    out=ex[:, : 2 * iw],
                    in_=sc[:, : 2 * iw],
                    func=Exp,
                    bias=ebias_sb,
                    scale=0.125,
                )
                if g == 1 and pending_tail is not None:
                    pending_tail()
                    pending_tail = None
                if prev_pv is not None:
                    pex, pg = prev_pv
                    for u in range(2):
                        t = 2 * pg + u
                        nc.tensor.matmul(
                            pv[:, :iw],
                            lhsT=v_sb[:, t, :],
                            rhs=pex[:, u * iw : (u + 1) * iw],
                            start=(t == 0),
                            stop=False,
                        )
                prev_pv = (ex, g)
            pex, pg = prev_pv
            for u in range(2):
                t = 2 * pg + u
                nc.tensor.matmul(
                    pv[:, :iw],
                    lhsT=v_sb[:, t, :],
                    rhs=pex[:, u * iw : (u + 1) * iw],
                    start=False,
                    stop=(t == NT - 1),
                )
            res_sb = resp.tile([DK + 1, 512], fr, tag="res", name="res_sb")
            nc.vector.tensor_copy(res_sb[:, :iw], pv[:, :iw])
            nc.sync.dma_start(
                out=lsum[0:1, ioff : ioff + iw],
                in_=res_sb[DK : DK + 1, :iw].bitcast(f32),
            )

            def tail(ioff=ioff, iw=iw, res_sb=res_sb):
                for cc in range(2):
                    po = psp.tile([128, 512], f32, tag="ps", name="po")
                    nc.tensor.matmul(
                        po[:, :iw],
                        lhsT=wo_sb[:, cc * 128 : (cc + 1) * 128],
                        rhs=res_sb[:DK, :iw],
                        start=True,
                        stop=True,
                    )
                    ob = outp.tile([128, 512], f32, tag="ob", name="ob")
                    nc.vector.tensor_copy(ob[:, :iw], po[:, :iw])
                    nc.sync.dma_start(
                        out=out[cc * 128 : (cc + 1) * 128, ioff : ioff + iw],
                        in_=ob[:, :iw],
                    )

            pending_tail = tail
        pending_tail()

    nc.compile()
    return nc


def _get_nc():
    global _NC
    if _NC is None:
        _NC = _build()
    return _NC


def kernel(x, w_proj, b_proj, w_out, b_out):
    from concourse.bass_utils import run_bass_kernel_spmd

    x = np.asarray(x, dtype=np.float32)
    w_proj = np.asarray(w_proj, dtype=np.float32)
    b_proj = np.asarray(b_proj, dtype=np.float32)
    w_out = np.asarray(w_out, dtype=np.float32)
    b_out = np.asarray(b_out, dtype=np.float32)

    B = x.shape[0]
    nc = _get_nc()

    in_maps = []
    for core in range(8):
        b, h = divmod(core, H)
        base = h * 3 * DK
        in_maps.append(
            {
                "xT": np.ascontiguousarray(x[b].reshape(C, S)),
                "wq": np.ascontiguousarray(w_proj[:, base : base + DK]),
                "wk": np.ascontiguousarray(w_proj[:, base + DK : base + 2 * DK]),
                "wv": np.ascontiguousarray(w_proj[:, base + 2 * DK : base + 3 * DK]),
                "bq": np.ascontiguousarray(b_proj[base : base + DK].reshape(DK, 1)),
                "bk": np.ascontiguousarray(
                    b_proj[base + DK : base + 2 * DK].reshape(DK, 1)
                ),
                "wo": np.ascontiguousarray(w_out[h * DK : (h + 1) * DK, :]),
            }
        )

    res = run_bass_kernel_spmd(nc, in_maps, list(range(8)))

    outs = np.zeros((B, C, S), dtype=np.float32)
    for b in range(B):
        acc = x[b].reshape(C, S).astype(np.float32) + b_out[:, None]
        for h in range(H):
            core = b * H + h
            dev_o = res.results[core]["out"]  # [C, S] unnormalized
            l = res.results[core]["lsum"]  # [1, S]
            bv = b_proj[h * 3 * DK + 2 * DK : h * 3 * DK + 3 * DK]
            corr = bv @ w_out[h * DK : (h + 1) * DK, :]  # [C]
            acc = acc + dev_o / l + corr[:, None]
        outs[b] = acc
    return outs.reshape(B, C, 14, 14, 14)


# revision 14
# speedup vs baseline: 2.6882x; 1.1041x over previous
"""AttentionBlock kernel for 8 Trainium2 NeuronCores.

Sharding: one (batch, head) pair per core (B=2 x H=4 = 8 cores).
Each core computes, for its (b, h):
    qT/kT = (w_q/k^T @ x_b) + bias        [64, S]   (S padded to 2816)
    v     = x_b^T @ w_v                   [S, 64]  (+ ones column -> [S, 65])
    S^T[j, i] = sum_d k[j,d] q[i,d]                (22 j-tiles of 128)
    E = exp(S^T * 0.125 - 3)                       (ScalarE, from PSUM)
    resT[d, i] = sum_j v_aug[j, d] E[j, i]         (PSUM accumulation, 65 rows;
                                                    row 64 = softmax denominator l)
    outT[c, i] = sum_d w_out[d, c] resT[d, i]      [256, S] (unnormalized)
Host: out_b = sum_h (outT / l + (b_v @ w_out_h)[:, None]) + b_out[:, None] + x_b.

The -3 bias and missing max-subtraction cancel in the normalization; score
scale is 1/sqrt(64) = 0.125.

Matmul operands use float32r (single-pass PE fp32, ~TF32 precision, ~3x the
throughput of 2-pass fp32); PSUM accumulation stays fp32.
"""

import numpy as np

C = 256
S = 2744
SP = 2816  # 22 * 128
H = 4
DK = 64
NT = 22  # j tiles of 128
SVALID_LAST = S - 21 * 128  # 56 valid rows in last j-tile

# i blocks (query positions): only valid range [0, 2744)
IBLOCKS = [(0, 512), (512, 512), (1024, 512), (1536, 512), (2048, 512), (2560, 184)]
# s blocks for the qk projection: full padded range [0, 2816)
SBLOCKS = [(0, 512), (512, 512), (1024, 512), (1536, 512), (2048, 512), (2560, 256)]

_NC = None
PACK_SCORES = True


def _build():
    from contextlib import ExitStack

    import concourse.bacc as bacc
    import concourse.tile as tile
    from concourse import mybir

    f32 = mybir.dt.float32
    fr = mybir.dt.float32r
    f16 = mybir.dt.float16
    Exp = mybir.ActivationFunctionType.Exp

    nc = bacc.Bacc("TRN2", target_bir_lowering=False)

    xT = nc.dram_tensor("xT", [C, S], f16, kind="ExternalInput")
    wq = nc.dram_tensor("wq", [C, DK], f16, kind="ExternalInput")
    wk = nc.dram_tensor("wk", [C, DK], f16, kind="ExternalInput")
    wv = nc.dram_tensor("wv", [C, DK], f16, kind="ExternalInput")
    bq = nc.dram_tensor("bq", [128, 1], f32, kind="ExternalInput")
    bk = nc.dram_tensor("bk", [128, 1], f32, kind="ExternalInput")
    wo = nc.dram_tensor("wo", [DK, C], f32, kind="ExternalInput")

    out = nc.dram_tensor("out", [C, S], f32, kind="ExternalOutput")
    lsum = nc.dram_tensor("lsum", [1, S], f32, kind="ExternalOutput")

    with tile.TileContext(nc) as tc, ExitStack() as ctx:
        consts = ctx.enter_context(tc.tile_pool(name="consts", bufs=1))
        big = ctx.enter_context(tc.tile_pool(name="big", bufs=1))
        expp = ctx.enter_context(tc.tile_pool(name="expp", bufs=3))
        resp = ctx.enter_context(tc.tile_pool(name="resp", bufs=2))
        outp = ctx.enter_context(tc.tile_pool(name="outp", bufs=2))
        scp = ctx.enter_context(tc.tile_pool(name="scp", bufs=2, space="PSUM"))
        psp = ctx.enter_context(tc.tile_pool(name="psp", bufs=4, space="PSUM"))

        # ---- weights / constants in SBUF (fp16 direct) ----
        w_sb = consts.tile([128, 2, 3 * DK], f16)
        for idx, w_dram in enumerate((wq, wk, wv)):
            nc.sync.dma_start(
                out=w_sb[:, :, idx * DK : (idx + 1) * DK],
                in_=w_dram.rearrange("(c p) d -> p c d", p=128),
            )

        def wslice(idx, cc):
            return w_sb[:, cc, idx * DK : (idx + 1) * DK]

        wo_stage = consts.tile([DK, C], f32)
        nc.sync.dma_start(out=wo_stage, in_=wo[:, :])
        wo_sb = consts.tile([DK, C], fr)
        nc.vector.tensor_copy(wo_sb, wo_stage)

        bq_sb = consts.tile([128, 1], f32)
        nc.sync.dma_start(out=bq_sb, in_=bq[:, :])
        bk_sb = consts.tile([128, 1], f32)
        nc.sync.dma_start(out=bk_sb, in_=bk[:, :])
        ebias_sb = consts.tile([128, 1], f32)
        nc.vector.memset(ebias_sb, -3.0)

        # ---- x in SBUF (fp16 direct) ----
        x_sb = big.tile([128, 2, SP], f16)
        nc.vector.memset(x_sb[:, :, S:SP], 0.0)
        for cc in range(2):
            for off, w in SBLOCKS:
                wv_ = min(w, S - off) if off < S else 0
                if wv_ > 0:
                    nc.sync.dma_start(
                        out=x_sb[:, cc, off : off + wv_],
                        in_=xT[cc * 128 : (cc + 1) * 128, off : off + wv_],
                    )

        # ---- q/k projections -> qT_sb/kT_sb [64, SP] f32r (bias added) ----
        qT_sb = big.tile([128, SP], f16)
        kT_sb = big.tile([128, SP], f16)
        for off, w in SBLOCKS:
            psq = psp.tile([DK, 512], f32, tag="ps", name="psq")
            for cc in range(2):
                nc.tensor.matmul(
                    psq[:, :w],
                    lhsT=wslice(0, cc),
                    rhs=x_sb[:, cc, off : off + w],
                    start=(cc == 0),
                    stop=(cc == 1),
                )
            nc.vector.tensor_scalar_add(
                qT_sb[:DK, off : off + w], psq[:, :w], bq_sb[:DK]
            )
            nc.sync.dma_start(
                out=qT_sb[DK:, off : off + w], in_=qT_sb[:DK, off : off + w]
            )
            psk = psp.tile([DK, 512], f32, tag="ps", name="psk")
            for cc in range(2):
                nc.tensor.matmul(
                    psk[:, :w],
                    lhsT=wslice(1, cc),
                    rhs=x_sb[:, cc, off : off + w],
                    start=(cc == 0),
                    stop=(cc == 1),
                )
            nc.vector.tensor_scalar_add(
                kT_sb[:DK, off : off + w], psk[:, :w], bk_sb[:DK]
            )
            nc.sync.dma_start(
                out=kT_sb[DK:, off : off + w], in_=kT_sb[:DK, off : off + w]
            )

        # ---- v projection -> v_sb [128, NT, 65] f32r; col 64 = ones ----
        v_sb = big.tile([128, NT, DK + 1], f16)
        nc.vector.memset(v_sb[:, : NT - 1, DK : DK + 1], 1.0)
        nc.vector.memset(v_sb[:, NT - 1, DK : DK + 1], 0.0)
        nc.vector.memset(v_sb[:SVALID_LAST, NT - 1, DK : DK + 1], 1.0)
        for t in range(NT):
            psv = psp.tile([128, DK], f32, tag="ps", name="psv")
            for cc in range(2):
                nc.tensor.matmul(
                    psv,
                    lhsT=x_sb[:, cc, t * 128 : (t + 1) * 128],
                    rhs=wslice(2, cc),
                    start=(cc == 0),
                    stop=(cc == 1),
                )
            nc.vector.tensor_copy(v_sb[:, t, :DK], psv)

        # ---- main attention loop ----
        NG = NT // 2  # groups of 2 j-tiles per exp op
        pending_tail = None
        for ioff, iw in IBLOCKS:
            pv = psp.tile([DK + 1, 512], f32, tag="ps", name="pv")
            prev_pv = None  # (ex tile, group index) pending PV matmuls
            for g in range(NG):
                sc = scp.tile([128, 1024], f32, tag="sc", name="sc")
                for u in range(2):
                    t = 2 * g + u
                    lo, hi = (u * DK, (u + 1) * DK) if PACK_SCORES else (0, DK)
                    nc.tensor.matmul(
                        sc[:, u * 512 : u * 512 + iw],
                        lhsT=kT_sb[lo:hi, t * 128 : (t + 1) * 128],
                        rhs=qT_sb[lo:hi, ioff : ioff + iw],
                        start=True,
                        stop=True,
                        tile_position=(lo, 0),
                    )
                ex = expp.tile([128, 1024], f16, tag="ex", name="ex")
                sc3 = sc.rearrange("p (b w) -> p b w", b=2)[:, :, :iw]
                ex3 = ex.rearrange("p (b w) -> p b w", b=2)[:, :, :iw]
                nc.scalar.activation(
                    out=ex3,
                    in_=sc3,
                    func=Exp,
                    bias=ebias_sb,
                    scale=0.125,
                )
                if g == 1 and pending_tail is not None:
                    pending_tail()
                    pending_tail = None
                if prev_pv is not None:
                    pex, pg = prev_pv
                    for u in range(2):
                        t = 2 * pg + u
                        nc.tensor.matmul(
                            pv[:, :iw],
                            lhsT=v_sb[:, t, :],
                            rhs=pex[:, u * 512 : u * 512 + iw],
                            start=(t == 0),
                            stop=False,
                        )
                prev_pv = (ex, g)
            pex, pg = prev_pv
            for u in range(2):
                t = 2 * pg + u
                nc.tensor.matmul(
                    pv[:, :iw],
                    lhsT=v_sb[:, t, :],
                    rhs=pex[:, u * 512 : u * 512 + iw],
                    start=False,
                    stop=(t == NT - 1),
                )
            res_sb = resp.tile([DK + 1, 512], fr, tag="res", name="res_sb")
            nc.vector.tensor_copy(res_sb[:, :iw], pv[:, :iw])
            nc.sync.dma_start(
                out=lsum[0:1, ioff : ioff + iw],
                in_=res_sb[DK : DK + 1, :iw].bitcast(f32),
            )

            def tail(ioff=ioff, iw=iw, res_sb=res_sb):
                for cc in range(2):
                    po = psp.tile([128, 512], f32, tag="ps", name="po")
                    nc.tensor.matmul(
                        po[:, :iw],
                        lhsT=wo_sb[:, cc * 128 : (cc + 1) * 128],
                        rhs=res_sb[:DK, :iw],
                        start=True,
                        stop=True,
                    )
                    ob = outp.tile([128, 512], f32, tag="ob", name="ob")
                    nc.vector.tensor_copy(ob[:, :iw], po[:, :iw])
                    nc.sync.dma_start(
                        out=out[cc * 128 : (cc + 1) * 128, ioff : ioff + iw],
                        in_=ob[:, :iw],
                    )

            pending_tail = tail
        pending_tail()

    nc.compile()
    return nc


def _get_nc():
    global _NC
    if _NC is None:
        _NC = _build()
    return _NC



def _make_in_maps(inputs):
    x = np.asarray(inputs["x"], dtype=np.float32)
    w_proj = np.asarray(inputs["w_proj"], dtype=np.float32)
    b_proj = np.asarray(inputs["b_proj"], dtype=np.float32)
    w_out = np.asarray(inputs["w_out"], dtype=np.float32)
    in_maps = []
    for core in range(8):
        b, h = divmod(core, H)
        base = h * 3 * DK
        in_maps.append(
            {
                "xT": np.ascontiguousarray(x[b].reshape(C, S).astype(np.float16)),
                "wq": np.ascontiguousarray(
                    w_proj[:, base : base + DK].astype(np.float16)
                ),
                "wk": np.ascontiguousarray(
                    w_proj[:, base + DK : base + 2 * DK].astype(np.float16)
                ),
                "wv": np.ascontiguousarray(
                    w_proj[:, base + 2 * DK : base + 3 * DK].astype(np.float16)
                ),
                "bq": np.ascontiguousarray(
                    np.tile(b_proj[base : base + DK], 2).reshape(128, 1)
                ),
                "bk": np.ascontiguousarray(
                    np.tile(b_proj[base + DK : base + 2 * DK], 2).reshape(128, 1)
                ),
                "wo": np.ascontiguousarray(w_out[h * DK : (h + 1) * DK, :]),
            }
        )
    return in_maps


def kernel(x, w_proj, b_proj, w_out, b_out):
    from concourse.bass_utils import run_bass_kernel_spmd

    x = np.asarray(x, dtype=np.float32)
    w_proj = np.asarray(w_proj, dtype=np.float32)
    b_proj = np.asarray(b_proj, dtype=np.float32)
    w_out = np.asarray(w_out, dtype=np.float32)
    b_out = np.asarray(b_out, dtype=np.float32)

    B = x.shape[0]
    nc = _get_nc()

    in_maps = _make_in_maps(
        {"x": x, "w_proj": w_proj, "b_proj": b_proj, "w_out": w_out, "b_out": b_out}
    )
    res = run_bass_kernel_spmd(nc, in_maps, list(range(8)))

    outs = np.zeros((B, C, S), dtype=np.float32)
    for b in range(B):
        acc = x[b].reshape(C, S).astype(np.float32) + b_out[:, None]
        for h in range(H):
            core = b * H + h
            dev_o = res.results[core]["out"]  # [C, S] unnormalized
            l = res.results[core]["lsum"]  # [1, S]
            bv = b_proj[h * 3 * DK + 2 * DK : h * 3 * DK + 3 * DK]
            corr = bv @ w_out[h * DK : (h + 1) * DK, :]  # [C]
            acc = acc + dev_o / l + corr[:, None]
        outs[b] = acc
    return outs.reshape(B, C, 14, 14, 14)
